# revision 27
# baseline (speedup 1.0000x reference)
"""AttnBlock on 8 trn2 cores — merged-projection + fp8 DoubleRow variant.

Algebraic reductions vs the 4-projection form (all exact):
  scores: q^T k = h^T (wk^T wq) h + u[key] + (per-query terms that cancel in
          softmax), with u = (wk^T bq)^T h added as the per-partition exp bias.
          -> ONE projection mh = (wq^T wk)^T h instead of q and k.
  output: wo @ (P V)/rowsum = (P (wo wv h))/rowsum -> value projection uses
          wov = wo @ wv directly; the output-projection matmuls disappear.
          bv folds into bo' = bo + wo @ bv on the host (softmax rows sum to 1).

fp8: h/mh/vot/weights in e4m3 (TRN max 240), exp(S) in e5m2 (range to 57344,
no max-subtraction needed), all attention-phase matmuls use DoubleRow perf
mode (2 fp8 k-tiles per instruction ~= 2x bf16 column rate). PSUM stays f32.
Simulated end-to-end rel err ~2.5e-3 vs the 2e-2 gate.

Per-core: GroupNorm (chunk-streamed stats, block-diagonal group matmul),
mh/voT/u projections (DoubleRow), then S^T tiles -> exp(e5m2) -> PV +
ones-rowsum DoubleRow matmuls, normalize + bias + residual per query block.
x stays SBUF-resident from phase 1 (no re-DMA for the residual).
"""

import numpy as np
import ml_dtypes

C = 512
N = 4096
NT = 4
NP = 2           # c-chunk pairs for DoubleRow
BLK = 512
NB = N // BLK
NJ = N // 128
NJ2 = NJ // 2    # j-chunk pairs for DoubleRow PV
GROUP = 16
EPS = 1e-5
SCALE = float(C) ** -0.5
NCORES = 8
HW = 64

# Schraudolph exp on DVE for a few pairs per query block (rebalances the
# ACT-bound exp stream): exp(s*SCALE) ~= bitcast_f32(int32(s*SCH_A + SCH_B))
DVE_EXP_J2 = frozenset({4, 9, 13})
SCH_A = (1 << 23) / float(np.log(2.0)) * SCALE
SCH_B = float((127 << 23) - int(0.0436 * (1 << 23)))

BF16 = ml_dtypes.bfloat16
E4 = ml_dtypes.float8_e4m3
E5 = ml_dtypes.float8_e5m2

_cache = {}


def _build(n_repeat=1, with_u=False):
    import concourse.bacc as bacc
    import concourse.mybir as mybir
    import concourse.tile as tile
    from contextlib import ExitStack

    f32 = mybir.dt.float32
    f8e4 = mybir.dt.float8e4
    f8e5 = mybir.dt.float8e5
    AF = mybir.ActivationFunctionType
    OP = mybir.AluOpType
    AX = mybir.AxisListType
    DR = mybir.MatmulPerfMode.DoubleRow

    nc = bacc.Bacc(
        "TRN2",
        target_bir_lowering=False,
        debug=False,
        enable_asserts=False,
        num_devices=NCORES,
    )

    bf16 = mybir.dt.bfloat16
    wu_d = None
    x_d = nc.dram_tensor("x", [C, N], bf16, kind="ExternalInput")
    m1T_d = [
        nc.dram_tensor(f"m1T8_{p}", [128, 2 * C], f8e4, kind="ExternalInput")
        for p in range(NP)
    ]
    wovT_d = [
        nc.dram_tensor(f"wovT8_{p}", [128, 2 * C], f8e4, kind="ExternalInput")
        for p in range(NP)
    ]
    if with_u:
        wu_d = nc.dram_tensor("wu8_t", [128, NT], f8e4, kind="ExternalInput")
    bo2_d = nc.dram_tensor("bo2_t", [128, NT], f32, kind="ExternalInput")
    gnw_d = nc.dram_tensor("gnw_t", [128, NT], f32, kind="ExternalInput")
    gnb_d = nc.dram_tensor("gnb_t", [128, NT], f32, kind="ExternalInput")
    ones_d = nc.dram_tensor("ones8", [128, 2 * 128], f8e5, kind="ExternalInput")
    mgrp_d = nc.dram_tensor("mgrp", [128, 128], f32, kind="ExternalInput")
    out_d = nc.dram_tensor("out", [C, N], f32, kind="ExternalOutput")

    with tile.TileContext(nc) as tc:
        for rep in range(n_repeat):
            with ExitStack() as ctx:
                persist = ctx.enter_context(
                    tc.tile_pool(name=f"persist{rep}", bufs=1)
                )

                ones_sb = persist.tile([128, 2, 128], f8e5, name="ones_sb")
                nc.sync.dma_start(ones_sb[:], ones_d.ap())
                mgrp_sb = persist.tile([128, 128], f32, name="mgrp_sb")
                nc.sync.dma_start(mgrp_sb[:], mgrp_d.ap())
                if with_u:
                    wu_sb = persist.tile([128, NT], f8e4, name="wu_sb")
                    nc.sync.dma_start(wu_sb[:], wu_d.ap())
                bo2_sb = persist.tile([128, NT], f32, name="bo2_sb")
                nc.sync.dma_start(bo2_sb[:], bo2_d.ap())
                gnw_sb = persist.tile([128, NT], f32, name="gnw_sb")
                nc.sync.dma_start(gnw_sb[:], gnw_d.ap())
                gnb_sb = persist.tile([128, NT], f32, name="gnb_sb")
                nc.sync.dma_start(gnb_sb[:], gnb_d.ap())

                m1T_sb = [
                    persist.tile([128, 2, C], f8e4, name=f"m1T{p}") for p in range(NP)
                ]
                wovT_sb = [
                    persist.tile([128, 2, C], f8e4, name=f"wovT{p}") for p in range(NP)
                ]

                h8 = [persist.tile([128, 2, N], f8e4, name=f"h8_{p}") for p in range(NP)]
                mh8 = [
                    persist.tile([128, 2, N], f8e4, name=f"mh8_{p}") for p in range(NP)
                ]
                vot8 = [
                    persist.tile([128, 2, BLK], f8e4, name=f"vot8_{j2}")
                    for j2 in range(NJ2)
                ]
                us_sb = persist.tile([128, NJ], f32, name="us_sb")

                stats = persist.tile([128, 8 * NT], f32, name="stats")
                a_t = persist.tile([128, NT], f32, name="a_t")
                b_t = persist.tile([128, NT], f32, name="b_t")
                eps_sb = persist.tile([128, 1], f32, name="eps_sb")
                nc.vector.memset(eps_sb[:], EPS)

                from contextlib import ExitStack as _ES
                xctx = _ES()
                xpool = xctx.enter_context(tc.tile_pool(name=f"xpool{rep}", bufs=1))

                # ---------------- Phase 1: GroupNorm statistics ----------------
                xq = [[None] * 4 for _ in range(NT)]
                with tc.tile_pool(name="scr", bufs=3) as scrp, tc.tile_pool(
                    name="psg", bufs=1, space="PSUM"
                ) as psg, tc.tile_pool(name="warm", bufs=1, space="PSUM") as wrm:
                    warm_ps = wrm.tile([128, BLK], f32, name="warm_ps")
                    for c in range(NT):
                        for ch in range(4):
                            xt = xpool.tile([128, 1024], bf16, name=f"x_{c}_{ch}")
                            nc.sync.dma_start(
                                xt[:],
                                x_d.ap()[
                                    c * 128 : (c + 1) * 128,
                                    ch * 1024 : (ch + 1) * 1024,
                                ],
                            )
                            xq[c][ch] = xt
                            col = 4 * c + ch
                            nc.vector.reduce_sum(
                                stats[:, col : col + 1], xt[:], axis=AX.X
                            )
                            scr = scrp.tile([128, 1024], f32, tag="scr", name="scr")
                            nc.scalar.activation(
                                scr[:],
                                xt[:],
                                AF.Square,
                                accum_out=stats[:, 16 + col : 16 + col + 1],
                            )
                            # PE-clock warmer gated on this chunk's DMA
                            nc.tensor.matmul(
                                warm_ps[:],
                                xt[:, 0:128],
                                xt[:, 0:BLK],
                                start=True,
                                stop=True,
                            )
                    for p in range(NP):
                        nc.sync.dma_start(m1T_sb[p][:], m1T_d[p].ap())
                        nc.sync.dma_start(wovT_sb[p][:], wovT_d[p].ap())
                    psG = psg.tile([128, 8 * NT], f32, name="psG")
                    nc.tensor.matmul(
                        psG[:], mgrp_sb[:], stats[:], start=True, stop=True
                    )
                    m2c = persist.tile([128, 2 * NT], f32, name="m2c")
                    nc.vector.reduce_sum(
                        m2c[:, 0:NT],
                        psG[:, 0:16].rearrange("p (a b) -> p a b", a=4),
                        axis=AX.X,
                    )
                    nc.vector.reduce_sum(
                        m2c[:, NT : 2 * NT],
                        psG[:, 16:32].rearrange("p (a b) -> p a b", a=4),
                        axis=AX.X,
                    )
                    m2 = persist.tile([128, 2 * NT], f32, name="m2")
                    nc.vector.tensor_scalar_mul(m2[:], m2c[:], 1.0 / (GROUP * N))
                    meansq = persist.tile([128, NT], f32, name="meansq")
                    nc.vector.tensor_mul(meansq[:], m2[:, 0:NT], m2[:, 0:NT])
                    var = persist.tile([128, NT], f32, name="var")
                    nc.vector.tensor_sub(var[:], m2[:, NT : 2 * NT], meansq[:])
                    sdev = persist.tile([128, NT], f32, name="sdev")
                    nc.scalar.activation(sdev[:], var[:], AF.Sqrt, bias=eps_sb[:])
                    rstd = persist.tile([128, NT], f32, name="rstd")
                    nc.vector.reciprocal(rstd[:], sdev[:])
                    nc.vector.tensor_mul(a_t[:], rstd[:], gnw_sb[:])
                    t1 = persist.tile([128, NT], f32, name="t1")
                    nc.vector.tensor_mul(t1[:], m2[:, 0:NT], a_t[:])
                    nc.vector.tensor_sub(b_t[:], gnb_sb[:], t1[:])

                # ---- Phase 2: normalize + mh / voT / u projections (fp8 DR) ----
                with tc.tile_pool(name="ps2", bufs=3, space="PSUM") as ps2, tc.tile_pool(
                    name="psu", bufs=2, space="PSUM"
                ) as psu:
                    copy_alt = [0]

                    def pcopy(dst, src):
                        # alternate PSUM->SBUF quantizing copies across ACT/DVE
                        if copy_alt[0] % 2 == 0:
                            nc.scalar.copy(dst, src)
                        else:
                            nc.vector.tensor_copy(dst, src)
                        copy_alt[0] += 1

                    for ch in range(4):
                        for c in range(NT):
                            # normalize split across the three non-PE engines
                            dst = h8[c // 2][
                                :, c % 2, ch * 1024 : (ch + 1) * 1024
                            ]
                            src = xq[c][ch][:]
                            if c < 2:
                                nc.gpsimd.tensor_scalar(
                                    dst, src,
                                    a_t[:, c : c + 1], b_t[:, c : c + 1],
                                    op0=OP.mult, op1=OP.add,
                                )
                            elif c == 2:
                                nc.scalar.activation(
                                    dst, src, AF.Identity,
                                    bias=b_t[:, c : c + 1],
                                    scale=a_t[:, c : c + 1],
                                )
                            else:
                                nc.vector.tensor_scalar(
                                    dst, src,
                                    a_t[:, c : c + 1], b_t[:, c : c + 1],
                                    op0=OP.mult, op1=OP.add,
                                )
                        for sub in range(2):
                            nb = 2 * ch + sub
                            sl = slice(nb * BLK, (nb + 1) * BLK)
                            for o2 in range(2):
                                qp2 = ps2.tile(
                                    [128, 2 * BLK], f32, tag="ps2", name="qp2"
                                )
                                for half in range(2):
                                    o4 = 2 * o2 + half
                                    hsl = slice(half * BLK, (half + 1) * BLK)
                                    for p in range(NP):
                                        nc.tensor.matmul(
                                            qp2[:, hsl],
                                            m1T_sb[p][
                                                :, :, o4 * 128 : (o4 + 1) * 128
                                            ],
                                            h8[p][:, :, sl],
                                            start=(p == 0),
                                            stop=(p == NP - 1),
                                            perf_mode=DR,
                                        )
                                pcopy(mh8[o2][:, :, sl], qp2[:])
                            for vh in range(2):
                                j2v = nb * 2 + vh
                                vp2 = ps2.tile(
                                    [128, 2 * BLK], f32, tag="ps2", name="vp2"
                                )
                                for half in range(2):
                                    j = nb * 4 + 2 * vh + half
                                    cols = slice(j * 128, (j + 1) * 128)
                                    hsl = slice(half * BLK, (half + 1) * BLK)
                                    for p in range(NP):
                                        nc.tensor.matmul(
                                            vp2[:, hsl],
                                            h8[p][:, :, cols],
                                            wovT_sb[p][:],
                                            start=(p == 0),
                                            stop=(p == NP - 1),
                                            perf_mode=DR,
                                        )
                                pcopy(vot8[j2v][:], vp2[:])
                                if with_u:
                                    for half in range(2):
                                        j = nb * 4 + 2 * vh + half
                                        cols = slice(j * 128, (j + 1) * 128)
                                        up = psu.tile(
                                            [128, 1], f32, tag="u", name="up"
                                        )
                                        for c in range(NT):
                                            nc.tensor.matmul(
                                                up[:],
                                                h8[c // 2][:, c % 2, cols],
                                                wu_sb[:, c : c + 1],
                                                start=(c == 0),
                                                stop=(c == NT - 1),
                                            )
                                        nc.vector.tensor_scalar_mul(
                                            us_sb[:, j : j + 1], up[:], SCALE
                                        )

                # ---- Phase 3: attention + normalize + bias + residual ----
                with tc.tile_pool(name="esp", bufs=18) as esp, tc.tile_pool(
                    name="i32p", bufs=2
                ) as i32p, tc.tile_pool(
                    name="pss", bufs=3 if with_u else 2, space="PSUM"
                ) as pss, tc.tile_pool(
                    name="pso", bufs=5 if with_u else 4, space="PSUM"
                ) as pso, tc.tile_pool(name="ph3", bufs=3) as ph3, tc.tile_pool(
                    name="tmp", bufs=10
                ) as tmpp, tc.tile_pool(name="opp", bufs=6) as opp:
                    def emit_S2(ib, j2):
                        sl = slice(ib * BLK, (ib + 1) * BLK)
                        pS = pss.tile([128, 2 * BLK], f32, tag="s", name="pS2")
                        for half in range(2):
                            jj = 2 * j2 + half
                            hsl = slice(half * BLK, (half + 1) * BLK)
                            for p in range(NP):
                                nc.tensor.matmul(
                                    pS[:, hsl],
                                    h8[p][:, :, jj * 128 : (jj + 1) * 128],
                                    mh8[p][:, :, sl],
                                    start=(p == 0),
                                    stop=(p == NP - 1),
                                    perf_mode=DR,
                                )
                        return pS

                    pS_cur = emit_S2(0, 0) if not with_u else None
                    for ib in range(NB):
                        sl = slice(ib * BLK, (ib + 1) * BLK)
                        pO = [
                            pso.tile([128, BLK], f32, tag="acc", name=f"pO{c4}")
                            for c4 in range(NT)
                        ]

                        if not with_u:
                            # fast path: S pairs -> one exp per [128,1024] pair,
                            # rowsum as a PE tail over the resident eS pairs,
                            # S software-pipelined across ib boundaries
                            eS_all = []
                            for j2 in range(NJ2):
                                if j2 + 1 < NJ2:
                                    pS_next = emit_S2(ib, j2 + 1)
                                    if j2 == NJ2 - 2:
                                        # next ib's first S pair, emitted BEFORE
                                        # the rowsum tail so pR lands on S15's
                                        # rotation slot, not S0''s
                                        pS_carry = (
                                            emit_S2(ib + 1, 0)
                                            if ib + 1 < NB
                                            else None
                                        )
                                else:
                                    pS_next = pS_carry
                                eS = esp.tile(
                                    [128, 2, BLK], f8e5, tag="es", name="eS"
                                )
                                if j2 in DVE_EXP_J2:
                                    i32 = i32p.tile(
                                        [128, 2 * BLK],
                                        mybir.dt.int32,
                                        tag="i",
                                        name="i32",
                                    )
                                    nc.vector.tensor_scalar(
                                        i32[:],
                                        pS_cur[:],
                                        SCH_A,
                                        SCH_B,
                                        op0=OP.mult,
                                        op1=OP.add,
                                    )
                                    nc.vector.tensor_copy(
                                        eS[:], i32[:].bitcast(f32)
                                    )
                                else:
                                    nc.scalar.activation(
                                        eS[:, :, :], pS_cur[:], AF.Exp, scale=SCALE
                                    )
                                eS_all.append(eS)
                                pS_cur = pS_next
                                if j2 == NJ2 - 1:
                                    # rowsum for pairs 0..14 is ready now (only
                                    # needs their exps); issue it on PE ahead of
                                    # the last PV group so it overlaps exp(15).
                                    # Shares the "s" rotation slots (pair tiles
                                    # are dead once exp'd).
                                    pR2 = pss.tile(
                                        [128, 2 * BLK], f32, tag="s", name="pR2"
                                    )
                                    pR = pR2[:, 0:BLK]
                                    for jr in range(NJ2 - 1):
                                        nc.tensor.matmul(
                                            pR,
                                            ones_sb[:],
                                            eS_all[jr][:],
                                            start=(jr == 0),
                                            stop=False,
                                            perf_mode=DR,
                                        )
                                for c4 in range(NT):
                                    nc.tensor.matmul(
                                        pO[c4][:],
                                        vot8[j2][:, :, c4 * 128 : (c4 + 1) * 128],
                                        eS[:],
                                        start=(j2 == 0),
                                        stop=(j2 == NJ2 - 1),
                                        perf_mode=DR,
                                    )
                            nc.tensor.matmul(
                                pR,
                                ones_sb[:],
                                eS_all[NJ2 - 1][:],
                                start=False,
                                stop=True,
                                perf_mode=DR,
                            )
                        else:
                            pR = pso.tile([128, BLK], f32, tag="acc", name="pR")

                            def emit_S(j):
                                pS = pss.tile([128, BLK], f32, tag="s", name="pS")
                                for p in range(NP):
                                    nc.tensor.matmul(
                                        pS[:],
                                        h8[p][:, :, j * 128 : (j + 1) * 128],
                                        mh8[p][:, :, sl],
                                        start=(p == 0),
                                        stop=(p == NP - 1),
                                        perf_mode=DR,
                                    )
                                return pS

                            pS_cur = emit_S(0)
                            eS = None
                            for j in range(NJ):
                                j2 = j // 2
                                pS_next = emit_S(j + 1) if j + 1 < NJ else None
                                if j % 2 == 0:
                                    eS = esp.tile(
                                        [128, 2, BLK], f8e5, tag="es", name="eS"
                                    )
                                nc.scalar.activation(
                                    eS[:, j % 2, :],
                                    pS_cur[:],
                                    AF.Exp,
                                    scale=SCALE,
                                    bias=us_sb[:, j : j + 1],
                                )
                                pS_cur = pS_next
                                if j % 2 == 1:
                                    for c4 in range(NT):
                                        nc.tensor.matmul(
                                            pO[c4][:],
                                            vot8[j2][:, :, c4 * 128 : (c4 + 1) * 128],
                                            eS[:],
                                            start=(j2 == 0),
                                            stop=(j2 == NJ2 - 1),
                                            perf_mode=DR,
                                        )
                                    nc.tensor.matmul(
                                        pR[:],
                                        ones_sb[:],
                                        eS[:],
                                        start=(j2 == 0),
                                        stop=(j2 == NJ2 - 1),
                                        perf_mode=DR,
                                    )
                            pR = pR[:]
                        recip = ph3.tile([128, BLK], f32, tag="recip", name="recip")
                        nc.vector.reciprocal_approx_fast(recip[:], pR)
                        for o4 in range(NT):
                            tmo = tmpp.tile([128, BLK], f32, tag="t", name="tmo")
                            nc.vector.tensor_mul(tmo[:], pO[o4][:], recip[:])
                            ot = opp.tile([128, BLK], f32, tag="op", name="ot")
                            xres = xq[o4][ib // 2][
                                :, (ib % 2) * BLK : (ib % 2) * BLK + BLK
                            ]
                            nc.vector.scalar_tensor_tensor(
                                ot[:],
                                tmo[:],
                                bo2_sb[:, o4 : o4 + 1],
                                xres,
                                op0=OP.add,
                                op1=OP.add,
                            )
                            nc.sync.dma_start(
                                out_d.ap()[o4 * 128 : (o4 + 1) * 128, sl], ot[:]
                            )

                xctx.close()

    nc.compile()
    return nc


def get_nc(n_repeat=1, with_u=False):
    key = (n_repeat, with_u)
    if key not in _cache:
        _cache[key] = _build(n_repeat, with_u)
    return _cache[key]


def _pair_pack(m):
    # [C, W] -> per-pair [128, 2, W]: tile[p][part, i, col] = m[(2p+i)*128+part, col]
    W = m.shape[1]
    out = []
    for p in range(NP):
        t = np.stack(
            [m[(2 * p) * 128 : (2 * p + 1) * 128], m[(2 * p + 1) * 128 : (2 * p + 2) * 128]],
            axis=1,
        )
        out.append(np.ascontiguousarray(t.reshape(128, 2 * W)))
    return out


def make_in_maps(x, gn_scale, gn_bias, wq, bq, wk, bk, wv, bv, wo, bo):
    B = x.shape[0]
    assert B == NCORES
    wq = np.asarray(wq, np.float32)
    wk = np.asarray(wk, np.float32)
    wv = np.asarray(wv, np.float32)
    wo = np.asarray(wo, np.float32)
    bq = np.asarray(bq, np.float32)
    bv = np.asarray(bv, np.float32)
    bo = np.asarray(bo, np.float32)
    # scores: q^T k = h^T (wk^T wq)... lhsT[c',c] = (wk^T wq)[c,c'] = (wq^T wk)[c',c]
    m1T = np.ascontiguousarray(wq.T @ wk)
    wovT = np.ascontiguousarray((wo @ wv).T)
    wu = wk.T @ bq
    bo2 = bo + wo @ bv

    m1T8 = [t.astype(E4) for t in _pair_pack(m1T)]
    wovT8 = [t.astype(E4) for t in _pair_pack(wovT)]

    def tile_vec(v):
        return np.ascontiguousarray(np.asarray(v, np.float32).reshape(NT, 128).T)

    shared = {
        "m1T8_0": m1T8[0],
        "m1T8_1": m1T8[1],
        "wovT8_0": wovT8[0],
        "wovT8_1": wovT8[1],
        "bo2_t": tile_vec(bo2),
        "gnw_t": tile_vec(gn_scale),
        "gnb_t": tile_vec(gn_bias),
        "ones8": np.ones((128, 2 * 128), E5),
        "mgrp": np.kron(
            np.eye(128 // GROUP, dtype=np.float32),
            np.ones((GROUP, GROUP), np.float32),
        ),
    }
    if np.any(wu != 0):
        # u = (wk^T bq) nonzero: per-key exp bias needed -> general path
        shared["wu8_t"] = tile_vec(wu).astype(E4)
    in_maps = []
    for i in range(B):
        m = dict(shared)
        m["x"] = np.ascontiguousarray(
            np.asarray(x[i], np.float32).reshape(C, N)
        ).astype(BF16)
        in_maps.append(m)
    return in_maps


def kernel(x, gn_scale, gn_bias, wq, bq, wk, bk, wv, bv, wo, bo):
    from concourse.bass_utils import run_bass_kernel_spmd

    in_maps = make_in_maps(x, gn_scale, gn_bias, wq, bq, wk, bk, wv, bv, wo, bo)
    nc = get_nc(1, with_u=("wu8_t" in in_maps[0]))
    res = run_bass_kernel_spmd(nc, in_maps, core_ids=list(range(NCORES)))
    out = np.stack(
        [res.results[i]["out"].reshape(C, HW, HW) for i in range(NCORES)]
    ).astype(np.float32)
    return out
